# revision 52
# baseline (speedup 1.0000x reference)
"""Trainium2 Bass kernel for nn_Classification_4922032521468.

Problem: acts = embeds[activity_index]  (A=512 rows, d=512)
         pairs = concat(acts[ii], acts[jj])  for all i<j (P=130816 pairs)
         out = log_softmax(pairs @ W.T + b)  -> [P, 4]

Algebra: logits[p, c] = L[i, c] + R'[j, c]  with
  L  = acts @ Wl.T          (Wl = W[:, :512])
  R' = acts @ Wr.T + b      (Wr = W[:, 512:])
log_softmax needs lse[j, i] = ln(sum_c e^{R'[j,c]} e^{L[i,c]}) (K=4 PE
matmul of V = e^{R'} against U = e^{L}) and
  out[j, i, c] = L[i, c] + R'[j, c] - lse[j, i].
No 130816x1024 pair tensor is ever built.

Fast path vs the 42us fp32 version:
 - fp16 data path end-to-end (PE 4x faster than fp32; DMA bytes halved;
   output cast back to fp32 on host). Headroom: harness gate is 2e-2,
   this lands ~1e-3.
 - ONE dma_gather(transpose=True): gathers all 512 rows AND transposes
   them into [128 d, 4 dchunk, 512 j] in a single SWDGE instruction -
   replaces 4 indirect gathers + 16 PE transposes + 16 PSUM copies.
 - R'^T and L^T fall out of 4 wide matmuls (N=512) into one PSUM bank.
 - One [8, 512] exp produces e^{R'+b} (rows 0:4) and e^{L} (rows 4:8).
 - L broadcast built without DMA hops: rhsL[c, (i,c')] = L[i,c]*I4[c,c']
   via one DVE op, then ones4^T @ rhsL = L broadcast to 128 partitions.
 - A manual LoadActFuncSet of a table set containing BOTH Exp and Ln is
   issued before any activation, so no ACT table switch sits on the
   critical path between exp and ln.
 - Fused fp16 combines ([128, 1024] in 2 DVE ops) and one output DMA.

Sharding: core k owns i-rows [64k, 64k+64). Same NEFF on all 8 cores;
per-core behavior via data only: activity_index rotated by -64k so core
k's i-rows are gathered rows 0..63. Each core outputs [512 j, 64 i, 4 c]
(j rotated); host un-rotates j, transposes, gathers the triu pairs.
"""

import numpy as np

A = 512  # number of activity tokens
D = 512  # embedding dim
C = 4  # classes
NTOK = 4096  # embeds table rows
RB = 64  # i-rows per core
NCORES = 8

_program = None
_last_results = None


def _build_program(debug_dump=False):
    from contextlib import ExitStack

    import concourse.bacc as bacc
    import concourse.mybir as mybir
    import concourse.tile as tile
    from concourse.bass import IndirectOffsetOnAxis
    from concourse.tile_rust import add_dep_helper

    fp32 = mybir.dt.float32
    fp16 = mybir.dt.float16
    i16 = mybir.dt.int16
    i32 = mybir.dt.int32
    AF = mybir.ActivationFunctionType
    SUB = mybir.AluOpType.subtract
    ADD = mybir.AluOpType.add
    MUL = mybir.AluOpType.mult

    nc = bacc.Bacc(
        "TRN2",
        target_bir_lowering=False,
        debug=False,
        enable_asserts=False,
        num_devices=NCORES,
    )

    emb16_h = nc.dram_tensor("emb16", (NTOK, D), fp16, kind="ExternalInput")
    # idxs[p, g] = rotated activity_index[128g + p], int32
    idx_h = nc.dram_tensor("idx32", (128, 4), i32, kind="ExternalInput")
    # wt[d, 8k+0:4] = Wr.T[128k+d, :], wt[d, 8k+4:8] = Wl.T[128k+d, :]
    wt_h = nc.dram_tensor("wt16", (128, 32), fp16, kind="ExternalInput")
    # b as a per-partition column
    b4_h = nc.dram_tensor("b4v", (C, 1), fp32, kind="ExternalInput")
    # out[j, 4i + c] fp16 (j rotated per core); host casts to fp32
    out_h = nc.dram_tensor("out", (A, RB * C), fp16, kind="ExternalOutput")

    i4_h = nc.inline_tensor(np.eye(C, dtype=np.float16), name="i4")
    id128_h = nc.inline_tensor(np.eye(128, dtype=np.float16), name="id128")

    with tile.TileContext(nc) as tc, ExitStack() as ctx:
        sb = ctx.enter_context(tc.tile_pool(name="sb", bufs=1))
        psA = ctx.enter_context(tc.tile_pool(name="psA", bufs=1, space="PSUM"))
        psB = ctx.enter_context(tc.tile_pool(name="psB", bufs=1, space="PSUM"))
        psC = ctx.enter_context(tc.tile_pool(name="psC", bufs=1, space="PSUM"))
        psD = ctx.enter_context(tc.tile_pool(name="psD", bufs=1, space="PSUM"))
        psE = ctx.enter_context(tc.tile_pool(name="psE", bufs=1, space="PSUM"))
        psT = ctx.enter_context(tc.tile_pool(name="psT", bufs=3, space="PSUM"))

        # ---- index load (critical path), then one indirect gather per
        # j-chunk: acts_g[p, :] = emb16[rot[128g + p], :] (no gpsimd ucode
        # library needed, contiguous full-speed row transfers) ----
        idxs = sb.tile([128, 4], i32, tag="idxs")
        nc.sync.dma_start(out=idxs[:], in_=idx_h.ap()[:])

        acts = []
        for g in range(4):
            ag = sb.tile([128, D], fp16, tag=f"acts{g}", name=f"acts{g}")
            nc.gpsimd.indirect_dma_start(
                out=ag[:],
                out_offset=None,
                in_=emb16_h.ap()[:],
                in_offset=IndirectOffsetOnAxis(ap=idxs[:, g : g + 1], axis=0),
            )
            acts.append(ag)

        # transpose on PE (g-major, as each gather lands), copies split
        # DVE/scalar, into k-major tiles aTk[d', 128g + jl]
        id128 = sb.tile([128, 128], fp16, tag="id128")
        nc.sync.dma_start(out=id128[:], in_=id128_h.ap()[:])
        aTk = [sb.tile([128, D], fp16, tag=f"aTk{k}", name=f"aTk{k}") for k in range(4)]
        for g in range(4):
            for k in range(4):
                pt = psT.tile([128, 128], fp16, tag="pt", name="pt")
                nc.tensor.transpose(
                    out=pt[:],
                    in_=acts[g][:, 128 * k : 128 * (k + 1)],
                    identity=id128[:],
                )
                if k < 3:
                    nc.vector.tensor_copy(
                        out=aTk[k][:, 128 * g : 128 * (g + 1)], in_=pt[:]
                    )
                else:
                    nc.scalar.activation(
                        out=aTk[k][:, 128 * g : 128 * (g + 1)],
                        in_=pt[:],
                        func=AF.Copy,
                    )

        # ---- small constants (off critical path) ----
        wt = sb.tile([128, 32], fp16, tag="wt")
        nc.scalar.dma_start(out=wt[:], in_=wt_h.ap()[:])
        b4v = sb.tile([C, 1], fp32, tag="b4v")
        nc.scalar.dma_start(out=b4v[:], in_=b4_h.ap()[:])
        i4 = sb.tile([C, C], fp16, tag="i4")
        nc.sync.dma_start(out=i4[:], in_=i4_h.ap()[:])
        ones4 = sb.tile([C, 128], fp16, tag="ones4")
        nc.vector.memset(ones4[:], 1.0)

        # ---- ACT table: load a set that has BOTH Exp and Ln, before any
        # activation, so no table switch lands mid-kernel ----
        load_inst = None
        try:
            from concourse.hw_specs import get_activation_tables

            tables = get_activation_tables(nc.m.arch)
            set_id = None
            for i, (name, funcs) in enumerate(tables.items()):
                if AF.Exp in funcs and AF.Ln in funcs:
                    set_id = i
                    break
            if set_id is not None:
                li = mybir.InstLoadActFuncSet(
                    name=nc.get_next_instruction_name(),
                    ins=[],
                    outs=[],
                    act_func_set_id=set_id,
                )
                load_inst = nc.scalar.add_instruction(li)
        except Exception:
            load_inst = None

        # ---- phase A: R'^T [4, 512] and L^T [4, 64], both base partition 0.
        # PE runs all matmul groups back-to-back; exp/bias-add per chunk
        # pipeline on scalar/DVE while later chunks are still in the PE. ----
        pr = psA.tile([C, D], fp32, tag="pr")
        pl = psE.tile([C, RB], fp32, tag="pl")
        uv = sb.tile([C, D + RB], fp16, tag="uv")
        rall = sb.tile([C, D], fp16, tag="rall")
        # L first (needs only aTk[:, 0:64] = chunk-0 columns)
        for k in range(4):
            nc.tensor.matmul(
                out=pl[:],
                lhsT=wt[:, 8 * k + 4 : 8 * k + 8],
                rhs=aTk[k][:, 0:RB],
                start=(k == 0),
                stop=(k == 3),
            )
        eu = nc.scalar.activation(out=uv[:, D : D + RB], in_=pl[:], func=AF.Exp)
        if load_inst is not None:
            add_dep_helper(eu.ins, load_inst.ins, sync=False, reason="act-table")

        # ---- L broadcast: lbb[p, (i, c)] = L[i, c] for all p ----
        rhsL = sb.tile([C, RB * C], fp16, tag="rhsL")
        nc.vector.tensor_tensor(
            out=rhsL[:].rearrange("c (i cc) -> c i cc", cc=C),
            in0=pl[:].unsqueeze(2).to_broadcast([C, RB, C]),
            in1=i4[:].unsqueeze(1).to_broadcast([C, RB, C]),
            op=MUL,
        )
        lbb = psB.tile([128, RB * C], fp32, tag="lbb")
        nc.tensor.matmul(out=lbb[:], lhsT=ones4[:], rhs=rhsL[:], start=True, stop=True)
        lbs = sb.tile([128, RB * C], fp16, tag="lbs")
        nc.vector.tensor_copy(out=lbs[:], in_=lbb[:])
        lbs3 = lbs[:].rearrange("p (i c) -> p i c", c=C)

        # ---- everything else in j-halves (h covers chunks 2h, 2h+1) so
        # half 0 combines and stores while half 1 is still gathering ----
        se = psD.tile([128, 4 * RB], fp32, tag="se")
        rjt = psC.tile([128, 4 * C], fp16, tag="rjt")
        rjs = sb.tile([128, 4 * C], fp16, tag="rjs")
        lnse = sb.tile([128, 4 * RB], fp16, tag="lnse")
        tall = sb.tile([128, 4 * RB * C], fp16, tag="tall")
        oall = sb.tile([128, 4 * RB * C], fp16, tag="oall")
        for h in range(2):
            jh = slice(256 * h, 256 * (h + 1))
            for k in range(4):
                nc.tensor.matmul(
                    out=pr[:, jh],
                    lhsT=wt[:, 8 * k : 8 * k + 4],
                    rhs=aTk[k][:, jh],
                    start=(k == 0),
                    stop=(k == 3),
                )
            nc.scalar.activation(
                out=uv[:, jh], in_=pr[:, jh], func=AF.Exp, bias=b4v[:]
            )
            nc.vector.tensor_scalar_add(rall[:, jh], pr[:, jh], b4v[:])
            for g in (2 * h, 2 * h + 1):
                nc.tensor.transpose(
                    out=rjt[:, C * g : C * (g + 1)],
                    in_=rall[:, 128 * g : 128 * (g + 1)],
                    identity=i4[:],
                )
                nc.tensor.matmul(
                    out=se[:, RB * g : RB * (g + 1)],
                    lhsT=uv[:, 128 * g : 128 * (g + 1)],
                    rhs=uv[:, D : D + RB],
                    start=True,
                    stop=True,
                )
            nc.vector.tensor_copy(
                out=rjs[:, 2 * C * h : 2 * C * (h + 1)],
                in_=rjt[:, 2 * C * h : 2 * C * (h + 1)],
            )
            nc.scalar.activation(
                out=lnse[:, 2 * RB * h : 2 * RB * (h + 1)],
                in_=se[:, 2 * RB * h : 2 * RB * (h + 1)],
                func=AF.Ln,
            )
            sl = slice(2 * RB * C * h, 2 * RB * C * (h + 1))
            nc.vector.tensor_tensor(
                out=tall[:, sl].rearrange("p (g i c) -> p g i c", g=2, c=C),
                in0=lbs3.unsqueeze(1).to_broadcast([128, 2, RB, C]),
                in1=rjs[:, 2 * C * h : 2 * C * (h + 1)]
                .rearrange("p (g c) -> p g c", g=2)
                .unsqueeze(2)
                .to_broadcast([128, 2, RB, C]),
                op=ADD,
            )
            nc.vector.tensor_tensor(
                out=oall[:, sl].rearrange("p (g i c) -> p g i c", g=2, c=C),
                in0=tall[:, sl].rearrange("p (g i c) -> p g i c", g=2, c=C),
                in1=lnse[:, 2 * RB * h : 2 * RB * (h + 1)]
                .rearrange("p (g i) -> p g i", g=2)
                .unsqueeze(3)
                .to_broadcast([128, 2, RB, C]),
                op=SUB,
            )
            # store half h: out rows 256h + 128g + p
            eng = nc.sync if h == 0 else nc.scalar
            eng.dma_start(
                out=out_h.ap()[256 * h : 256 * (h + 1), :].rearrange(
                    "(g p) f -> p g f", g=2
                ),
                in_=oall[:, sl].rearrange("p (g f) -> p g f", g=2),
            )

        if debug_dump:
            d_aT3 = nc.dram_tensor("d_aT3", (128, 4 * D), fp16, kind="ExternalOutput")
            for k in range(4):
                nc.sync.dma_start(
                    out=d_aT3.ap()[:, 512 * k : 512 * (k + 1)], in_=aTk[k][:]
                )
            d_uv = nc.dram_tensor("d_uv", (C, D + RB), fp16, kind="ExternalOutput")
            nc.sync.dma_start(out=d_uv.ap()[:], in_=uv[:])
            d_rall = nc.dram_tensor("d_rall", (C, D), fp16, kind="ExternalOutput")
            nc.sync.dma_start(out=d_rall.ap()[:], in_=rall[:])
            d_lnse = nc.dram_tensor("d_lnse", (128, 4 * RB), fp16, kind="ExternalOutput")
            nc.sync.dma_start(out=d_lnse.ap()[:], in_=lnse[:])
            d_rhsL = nc.dram_tensor("d_rhsL", (C, RB * C), fp16, kind="ExternalOutput")
            nc.sync.dma_start(out=d_rhsL.ap()[:], in_=rhsL[:])
            d_tall = nc.dram_tensor("d_tall", (128, 4 * RB * C), fp16, kind="ExternalOutput")
            nc.sync.dma_start(out=d_tall.ap()[:], in_=tall[:])

    nc.compile()
    return nc


def _get_program():
    global _program
    if _program is None:
        _program = _build_program()
    return _program


def _prep_core_inputs(emb16, idx64, wt_np, b4_np, k):
    rot = np.roll(idx64, -RB * k)
    idx32 = np.ascontiguousarray(rot.reshape(4, 128).T.astype(np.int32))
    return {"emb16": emb16, "idx32": idx32, "wt16": wt_np, "b4v": b4_np}


def kernel(embeds, activity_index, W, b):
    from concourse.bass_utils import run_bass_kernel_spmd

    emb16 = np.ascontiguousarray(np.asarray(embeds, dtype=np.float32).astype(np.float16))
    W = np.asarray(W, dtype=np.float32)
    b_in = np.asarray(b, dtype=np.float32).reshape(C)
    idx64 = np.asarray(activity_index).astype(np.int64)

    wt_np = np.empty((128, 32), dtype=np.float16)
    for k in range(4):
        wt_np[:, 8 * k : 8 * k + 4] = W[:, D + 128 * k : D + 128 * (k + 1)].T
        wt_np[:, 8 * k + 4 : 8 * k + 8] = W[:, 128 * k : 128 * (k + 1)].T
    wt_np = np.ascontiguousarray(wt_np)
    b4_np = np.ascontiguousarray(b_in.reshape(C, 1))

    nc = _get_program()
    in_maps = [
        _prep_core_inputs(emb16, idx64, wt_np, b4_np, k) for k in range(NCORES)
    ]

    results = run_bass_kernel_spmd(nc, in_maps, core_ids=list(range(NCORES)))
    global _last_results
    _last_results = results

    out_sq = np.empty((A, A, C), dtype=np.float32)
    for k in range(NCORES):
        blk = (
            results.results[k]["out"]
            .astype(np.float32)
            .reshape(A, RB, C)
            .transpose(1, 0, 2)
        )
        out_sq[RB * k : RB * (k + 1)] = np.roll(blk, RB * k, axis=1)

    ii, jj = np.triu_indices(A, k=1)
    return np.ascontiguousarray(out_sq[ii, jj])


# revision 53
# speedup vs baseline: 1.0272x; 1.0272x over previous
"""Trainium2 Bass kernel for nn_Classification_4922032521468.

Problem: acts = embeds[activity_index]  (A=512 rows, d=512)
         pairs = concat(acts[ii], acts[jj])  for all i<j (P=130816 pairs)
         out = log_softmax(pairs @ W.T + b)  -> [P, 4]

Algebra: logits[p, c] = L[i, c] + R'[j, c]  with
  L  = acts @ Wl.T          (Wl = W[:, :512])
  R' = acts @ Wr.T + b      (Wr = W[:, 512:])
log_softmax needs lse[j, i] = ln(sum_c e^{R'[j,c]} e^{L[i,c]}) (K=4 PE
matmul of V = e^{R'} against U = e^{L}) and
  out[j, i, c] = L[i, c] + R'[j, c] - lse[j, i].
No 130816x1024 pair tensor is ever built.

Fast path vs the 42us fp32 version (measured ~33us):
 - fp16 data path end-to-end (PE 4x faster than fp32; DMA bytes halved;
   output cast back to fp32 on host). Harness gate is 2e-2, this lands
   ~4e-4.
 - 4 indirect row gathers (fp16) pipeline into 16 PE transposes whose
   PSUM results are copied (DVE/scalar split) into k-major aTk tiles;
   R'^T then takes 4 wide (N=256) matmuls per j-half, L^T 4 narrow ones.
   (dma_gather(transpose=True) was tried and works but loses: ~9us mlp
   ucode library load + ~91GB/s XBAR-transposed transfers. A [128, 4]
   multi-offset indirect gather corrupts on HW - do not use.)
 - L broadcast built without DMA hops: rhsL[c, (i,c')] = L[i,c]*I4[c,c']
   via one DVE op, then ones4^T @ rhsL = L broadcast to 128 partitions.
 - A manual LoadActFuncSet of a table set containing BOTH Exp and Ln is
   issued before any activation, so no ACT table switch sits on the
   critical path between exp and ln.
 - Everything after the wide matmuls runs in j-halves (exp, +b staging,
   rjt transposes, se, ln, fused fp16 combines, store), so the first
   half's output DMA issues while the second half is still gathering.
Fixed overhead context: an empty 2-DMA NEFF spans ~18.5us on these
cores (preamble + queue init + teardown), so ~33us total = ~14us body.

Sharding: core k owns i-rows [64k, 64k+64). Same NEFF on all 8 cores;
per-core behavior via data only: activity_index rotated by -64k so core
k's i-rows are gathered rows 0..63. Each core outputs [512 j, 64 i, 4 c]
(j rotated); host un-rotates j, transposes, gathers the triu pairs.
"""

import numpy as np

A = 512  # number of activity tokens
D = 512  # embedding dim
C = 4  # classes
NTOK = 4096  # embeds table rows
RB = 64  # i-rows per core
NCORES = 8

_program = None
_last_results = None


def _build_program(debug_dump=False):
    from contextlib import ExitStack

    import concourse.bacc as bacc
    import concourse.mybir as mybir
    import concourse.tile as tile
    from concourse.bass import IndirectOffsetOnAxis
    from concourse.tile_rust import add_dep_helper

    fp32 = mybir.dt.float32
    fp16 = mybir.dt.float16
    i16 = mybir.dt.int16
    i32 = mybir.dt.int32
    AF = mybir.ActivationFunctionType
    SUB = mybir.AluOpType.subtract
    ADD = mybir.AluOpType.add
    MUL = mybir.AluOpType.mult

    nc = bacc.Bacc(
        "TRN2",
        target_bir_lowering=False,
        debug=False,
        enable_asserts=False,
        num_devices=NCORES,
    )

    emb16_h = nc.dram_tensor("emb16", (NTOK, D), fp16, kind="ExternalInput")
    # idxs[p, g] = rotated activity_index[128g + p], int32
    idx_h = nc.dram_tensor("idx32", (128, 4), i32, kind="ExternalInput")
    # wt[d, 8k+0:4] = Wr.T[128k+d, :], wt[d, 8k+4:8] = Wl.T[128k+d, :]
    wt_h = nc.dram_tensor("wt16", (128, 32), fp16, kind="ExternalInput")
    # b as a per-partition column
    b4_h = nc.dram_tensor("b4v", (C, 1), fp32, kind="ExternalInput")
    # out[j, 4i + c] fp16 (j rotated per core); host casts to fp32
    out_h = nc.dram_tensor("out", (A, RB * C), fp16, kind="ExternalOutput")

    i4_h = nc.inline_tensor(np.eye(C, dtype=np.float16), name="i4")
    id128_h = nc.inline_tensor(np.eye(128, dtype=np.float16), name="id128")

    with tile.TileContext(nc) as tc, ExitStack() as ctx:
        sb = ctx.enter_context(tc.tile_pool(name="sb", bufs=1))
        psA = ctx.enter_context(tc.tile_pool(name="psA", bufs=1, space="PSUM"))
        psB = ctx.enter_context(tc.tile_pool(name="psB", bufs=1, space="PSUM"))
        psC = ctx.enter_context(tc.tile_pool(name="psC", bufs=1, space="PSUM"))
        psD = ctx.enter_context(tc.tile_pool(name="psD", bufs=1, space="PSUM"))
        psE = ctx.enter_context(tc.tile_pool(name="psE", bufs=1, space="PSUM"))
        psT = ctx.enter_context(tc.tile_pool(name="psT", bufs=3, space="PSUM"))

        # ---- index load (critical path), then one indirect gather per
        # j-chunk: acts_g[p, :] = emb16[rot[128g + p], :] (no gpsimd ucode
        # library needed, contiguous full-speed row transfers) ----
        idxs = sb.tile([128, 4], i32, tag="idxs")
        nc.sync.dma_start(out=idxs[:], in_=idx_h.ap()[:])

        acts = []
        for g in range(4):
            ag = sb.tile([128, D], fp16, tag=f"acts{g}", name=f"acts{g}")
            nc.gpsimd.indirect_dma_start(
                out=ag[:],
                out_offset=None,
                in_=emb16_h.ap()[:],
                in_offset=IndirectOffsetOnAxis(ap=idxs[:, g : g + 1], axis=0),
            )
            acts.append(ag)

        # transpose on PE (g-major, as each gather lands), copies split
        # DVE/scalar, into k-major tiles aTk[d', 128g + jl]
        id128 = sb.tile([128, 128], fp16, tag="id128")
        nc.sync.dma_start(out=id128[:], in_=id128_h.ap()[:])
        aTk = [sb.tile([128, D], fp16, tag=f"aTk{k}", name=f"aTk{k}") for k in range(4)]
        for g in range(4):
            for k in range(4):
                pt = psT.tile([128, 128], fp16, tag="pt", name="pt")
                nc.tensor.transpose(
                    out=pt[:],
                    in_=acts[g][:, 128 * k : 128 * (k + 1)],
                    identity=id128[:],
                )
                if k < 3:
                    nc.vector.tensor_copy(
                        out=aTk[k][:, 128 * g : 128 * (g + 1)], in_=pt[:]
                    )
                else:
                    nc.scalar.activation(
                        out=aTk[k][:, 128 * g : 128 * (g + 1)],
                        in_=pt[:],
                        func=AF.Copy,
                    )

        # ---- small constants (off critical path) ----
        wt = sb.tile([128, 32], fp16, tag="wt")
        nc.scalar.dma_start(out=wt[:], in_=wt_h.ap()[:])
        b4v = sb.tile([C, 1], fp32, tag="b4v")
        nc.scalar.dma_start(out=b4v[:], in_=b4_h.ap()[:])
        i4 = sb.tile([C, C], fp16, tag="i4")
        nc.sync.dma_start(out=i4[:], in_=i4_h.ap()[:])
        ones4 = sb.tile([C, 128], fp16, tag="ones4")
        nc.vector.memset(ones4[:], 1.0)

        # ---- ACT table: load a set that has BOTH Exp and Ln, before any
        # activation, so no table switch lands mid-kernel ----
        load_inst = None
        try:
            from concourse.hw_specs import get_activation_tables

            tables = get_activation_tables(nc.m.arch)
            set_id = None
            for i, (name, funcs) in enumerate(tables.items()):
                if AF.Exp in funcs and AF.Ln in funcs:
                    set_id = i
                    break
            if set_id is not None:
                li = mybir.InstLoadActFuncSet(
                    name=nc.get_next_instruction_name(),
                    ins=[],
                    outs=[],
                    act_func_set_id=set_id,
                )
                load_inst = nc.scalar.add_instruction(li)
        except Exception:
            load_inst = None

        # ---- phase A: R'^T [4, 512] and L^T [4, 64], both base partition 0.
        # PE runs all matmul groups back-to-back; exp/bias-add per chunk
        # pipeline on scalar/DVE while later chunks are still in the PE. ----
        pr = psA.tile([C, D], fp32, tag="pr")
        pl = psE.tile([C, RB], fp32, tag="pl")
        uv = sb.tile([C, D + RB], fp16, tag="uv")
        rall = sb.tile([C, D], fp16, tag="rall")
        # L first (needs only aTk[:, 0:64] = chunk-0 columns)
        for k in range(4):
            nc.tensor.matmul(
                out=pl[:],
                lhsT=wt[:, 8 * k + 4 : 8 * k + 8],
                rhs=aTk[k][:, 0:RB],
                start=(k == 0),
                stop=(k == 3),
            )
        eu = nc.scalar.activation(out=uv[:, D : D + RB], in_=pl[:], func=AF.Exp)
        if load_inst is not None:
            add_dep_helper(eu.ins, load_inst.ins, sync=False, reason="act-table")

        # ---- L broadcast: lbb[p, (i, c)] = L[i, c] for all p ----
        rhsL = sb.tile([C, RB * C], fp16, tag="rhsL")
        nc.vector.tensor_tensor(
            out=rhsL[:].rearrange("c (i cc) -> c i cc", cc=C),
            in0=pl[:].unsqueeze(2).to_broadcast([C, RB, C]),
            in1=i4[:].unsqueeze(1).to_broadcast([C, RB, C]),
            op=MUL,
        )
        lbb = psB.tile([128, RB * C], fp32, tag="lbb")
        nc.tensor.matmul(out=lbb[:], lhsT=ones4[:], rhs=rhsL[:], start=True, stop=True)
        lbs = sb.tile([128, RB * C], fp16, tag="lbs")
        nc.vector.tensor_copy(out=lbs[:], in_=lbb[:])
        lbs3 = lbs[:].rearrange("p (i c) -> p i c", c=C)

        # ---- everything else in j-halves (h covers chunks 2h, 2h+1) so
        # half 0 combines and stores while half 1 is still gathering ----
        se = psD.tile([128, 4 * RB], fp32, tag="se")
        rjt = psC.tile([128, 4 * C], fp16, tag="rjt")
        rjs = sb.tile([128, 4 * C], fp16, tag="rjs")
        lnse = sb.tile([128, 4 * RB], fp16, tag="lnse")
        tall = sb.tile([128, 4 * RB * C], fp16, tag="tall")
        oall = sb.tile([128, 4 * RB * C], fp16, tag="oall")
        for h in range(2):
            jh = slice(256 * h, 256 * (h + 1))
            for k in range(4):
                nc.tensor.matmul(
                    out=pr[:, jh],
                    lhsT=wt[:, 8 * k : 8 * k + 4],
                    rhs=aTk[k][:, jh],
                    start=(k == 0),
                    stop=(k == 3),
                )
            nc.scalar.activation(
                out=uv[:, jh], in_=pr[:, jh], func=AF.Exp, bias=b4v[:]
            )
            nc.vector.tensor_scalar_add(rall[:, jh], pr[:, jh], b4v[:])
            for g in (2 * h, 2 * h + 1):
                nc.tensor.transpose(
                    out=rjt[:, C * g : C * (g + 1)],
                    in_=rall[:, 128 * g : 128 * (g + 1)],
                    identity=i4[:],
                )
                nc.tensor.matmul(
                    out=se[:, RB * g : RB * (g + 1)],
                    lhsT=uv[:, 128 * g : 128 * (g + 1)],
                    rhs=uv[:, D : D + RB],
                    start=True,
                    stop=True,
                )
            nc.vector.tensor_copy(
                out=rjs[:, 2 * C * h : 2 * C * (h + 1)],
                in_=rjt[:, 2 * C * h : 2 * C * (h + 1)],
            )
            nc.scalar.activation(
                out=lnse[:, 2 * RB * h : 2 * RB * (h + 1)],
                in_=se[:, 2 * RB * h : 2 * RB * (h + 1)],
                func=AF.Ln,
            )
            sl = slice(2 * RB * C * h, 2 * RB * C * (h + 1))
            nc.vector.tensor_tensor(
                out=tall[:, sl].rearrange("p (g i c) -> p g i c", g=2, c=C),
                in0=lbs3.unsqueeze(1).to_broadcast([128, 2, RB, C]),
                in1=rjs[:, 2 * C * h : 2 * C * (h + 1)]
                .rearrange("p (g c) -> p g c", g=2)
                .unsqueeze(2)
                .to_broadcast([128, 2, RB, C]),
                op=ADD,
            )
            nc.vector.tensor_tensor(
                out=oall[:, sl].rearrange("p (g i c) -> p g i c", g=2, c=C),
                in0=tall[:, sl].rearrange("p (g i c) -> p g i c", g=2, c=C),
                in1=lnse[:, 2 * RB * h : 2 * RB * (h + 1)]
                .rearrange("p (g i) -> p g i", g=2)
                .unsqueeze(3)
                .to_broadcast([128, 2, RB, C]),
                op=SUB,
            )
            # store half h: out rows 256h + 128g + p
            eng = nc.sync if h == 0 else nc.scalar
            eng.dma_start(
                out=out_h.ap()[256 * h : 256 * (h + 1), :].rearrange(
                    "(g p) f -> p g f", g=2
                ),
                in_=oall[:, sl].rearrange("p (g f) -> p g f", g=2),
            )

        if debug_dump:
            d_aT3 = nc.dram_tensor("d_aT3", (128, 4 * D), fp16, kind="ExternalOutput")
            for k in range(4):
                nc.sync.dma_start(
                    out=d_aT3.ap()[:, 512 * k : 512 * (k + 1)], in_=aTk[k][:]
                )
            d_uv = nc.dram_tensor("d_uv", (C, D + RB), fp16, kind="ExternalOutput")
            nc.sync.dma_start(out=d_uv.ap()[:], in_=uv[:])
            d_rall = nc.dram_tensor("d_rall", (C, D), fp16, kind="ExternalOutput")
            nc.sync.dma_start(out=d_rall.ap()[:], in_=rall[:])
            d_lnse = nc.dram_tensor("d_lnse", (128, 4 * RB), fp16, kind="ExternalOutput")
            nc.sync.dma_start(out=d_lnse.ap()[:], in_=lnse[:])
            d_rhsL = nc.dram_tensor("d_rhsL", (C, RB * C), fp16, kind="ExternalOutput")
            nc.sync.dma_start(out=d_rhsL.ap()[:], in_=rhsL[:])
            d_tall = nc.dram_tensor("d_tall", (128, 4 * RB * C), fp16, kind="ExternalOutput")
            nc.sync.dma_start(out=d_tall.ap()[:], in_=tall[:])

    nc.compile()
    return nc


def _get_program():
    global _program
    if _program is None:
        _program = _build_program()
    return _program


def _prep_core_inputs(emb16, idx64, wt_np, b4_np, k):
    rot = np.roll(idx64, -RB * k)
    idx32 = np.ascontiguousarray(rot.reshape(4, 128).T.astype(np.int32))
    return {"emb16": emb16, "idx32": idx32, "wt16": wt_np, "b4v": b4_np}


def kernel(embeds, activity_index, W, b):
    from concourse.bass_utils import run_bass_kernel_spmd

    emb16 = np.ascontiguousarray(np.asarray(embeds, dtype=np.float32).astype(np.float16))
    W = np.asarray(W, dtype=np.float32)
    b_in = np.asarray(b, dtype=np.float32).reshape(C)
    idx64 = np.asarray(activity_index).astype(np.int64)

    wt_np = np.empty((128, 32), dtype=np.float16)
    for k in range(4):
        wt_np[:, 8 * k : 8 * k + 4] = W[:, D + 128 * k : D + 128 * (k + 1)].T
        wt_np[:, 8 * k + 4 : 8 * k + 8] = W[:, 128 * k : 128 * (k + 1)].T
    wt_np = np.ascontiguousarray(wt_np)
    b4_np = np.ascontiguousarray(b_in.reshape(C, 1))

    nc = _get_program()
    in_maps = [
        _prep_core_inputs(emb16, idx64, wt_np, b4_np, k) for k in range(NCORES)
    ]

    results = run_bass_kernel_spmd(nc, in_maps, core_ids=list(range(NCORES)))
    global _last_results
    _last_results = results

    out_sq = np.empty((A, A, C), dtype=np.float32)
    for k in range(NCORES):
        blk = (
            results.results[k]["out"]
            .astype(np.float32)
            .reshape(A, RB, C)
            .transpose(1, 0, 2)
        )
        out_sq[RB * k : RB * (k + 1)] = np.roll(blk, RB * k, axis=1)

    ii, jj = np.triu_indices(A, k=1)
    return np.ascontiguousarray(out_sq[ii, jj])
